# revision 7
# baseline (speedup 1.0000x reference)
"""Trainium2 Bass kernel for nn_Attention_54305566490745 (pooling attention).

Algebraic reduction: the attention uses a single shared learned query per
head, so the whole module collapses to a weighted pooling:

    dots[b,h,n] = scale * ( x[b,:,n] . wq[:,h]  +  (q . pe)[h,n] )
    attn        = softmax_n(dots)
    s[b,h,:]    = sum_n attn[b,h,n] * x[b,:,n]           # pooled x
    out[b,h,:]  = s[b,h,:] @ Wv[:, h*64:(h+1)*64] + bv[h*64:(h+1)*64]

where wq[:,h] = Wk[:, h-block] @ q_h.

v2: single HBM read of x (c-major bf16 only, no transposed second copy).
The (n, c)-layout copy needed for the pooling contraction is produced
ON-CHIP: PE transposes x tiles into PSUM (stationary loads are free), and
Act/DVE alternate copying the PSUM tiles back to SBUF as bf16.  The dots
are computed TRANSPOSED (dotsT[n,h], x tiles stationary + tiny wq moving),
so exp runs on a [128, 64] tile and directly emits attnT — no separate
attention transpose.  Softmax sums are 1-column PE matmuls against ones;
normalization is deferred to the final projection output (per-partition
scale), fused with the bias add in one DVE scalar_tensor_tensor.

Distribution: data-parallel over batch, 8 batches per core on 8 cores.
HBM traffic per core = 8 MiB (one bf16 read of x) + 0.5 MiB Wv, which is
the DMA roofline for this problem at bf16.
"""

import math
import sys

sys.path.insert(0, "/opt/trn_rl_repo")

import numpy as np
import ml_dtypes

import concourse.bass as bass
import concourse.bacc as bacc
import concourse.mybir as mybir
from concourse import tile
from concourse.bass_utils import run_bass_kernel_spmd
from contextlib import ExitStack

BF16 = mybir.dt.bfloat16
F32 = mybir.dt.float32

B, D, HH, WW = 64, 512, 32, 32
N = HH * WW          # 1024
NH, DH = 8, 64
SCALE = DH ** -0.5
NCORES = 8
BPC = B // NCORES    # 8 batches per core
NCI = D // 128       # 4 c-chunks
NJ = N // 128        # 8 n-chunks


def _emit(ctx, tc, t):
    nc = tc.nc
    cst = ctx.enter_context(tc.tile_pool(name="cst", bufs=1))
    xn_pool = ctx.enter_context(tc.tile_pool(name="xn", bufs=3))
    xts_pool = ctx.enter_context(tc.tile_pool(name="xts", bufs=3))
    attn_pool = ctx.enter_context(tc.tile_pool(name="attn", bufs=3))
    tail_pool = ctx.enter_context(tc.tile_pool(name="tail", bufs=1))
    # PSUM: dt 2 + xt 4 + sT 1 + out 1 = 8 banks exactly
    dt_ps = ctx.enter_context(tc.tile_pool(name="dt_ps", bufs=2, space="PSUM"))
    xt_ps = ctx.enter_context(tc.tile_pool(name="xt_ps", bufs=4, space="PSUM"))
    st_ps = ctx.enter_context(tc.tile_pool(name="st_ps", bufs=1, space="PSUM"))
    out_ps = ctx.enter_context(tc.tile_pool(name="out_ps", bufs=1, space="PSUM"))

    # ---- constants (tiny, loaded first on the sync ring) ----
    wqpe = cst.tile([128, 8 * NCI], BF16, name="wqpe_sb")
    nc.sync.dma_start(wqpe[:], t["wqpe"])
    peqT = cst.tile([128, 8 * NJ], BF16, name="peqT_sb")
    nc.sync.dma_start(peqT[:], t["peqT"])
    i128 = cst.tile([128, 128], BF16, name="i128_sb")
    nc.sync.dma_start(i128[:], t["i128"])
    ones = cst.tile([128, 1], BF16, name="ones_sb")
    nc.vector.memset(ones[:], 1.0)
    nbias = cst.tile([128, 1], F32, name="nbias_sb")
    nc.vector.memset(nbias[:], -8.0)

    wv = cst.tile([128, NCI * D], BF16, name="wv_sb")
    bvr = cst.tile([64, D], F32, name="bvr_sb")
    rsum_all = cst.tile([64, 1], F32, name="rsum_all_sb")
    stsb = tail_pool.tile([128, BPC * 32], BF16, name="stsb")
    osb = tail_pool.tile([64, D], F32, name="osb")

    # s^T accumulator for all batches: [c(128), 64*ci + 8*b + h]
    st_acc = st_ps.tile([128, NCI * 64], F32, name="st_acc")
    ops = out_ps.tile([64, D], F32, name="out_psum")

    xb = t["xb"]

    # ---- x loads: b0/b7 as quarters (fill/drain), middle as halves;
    # even batches on the sync (HWDGE/SP) ring, odd on gpsimd (SWDGE/Pool)
    # so neither dispatch path saturates. ----
    xns = [None] * BPC

    def stage_load(b):
        xn = xn_pool.tile([128, NCI * N], BF16, name=f"xn{b}", tag="xn")
        xn3 = xn[:].rearrange("p (ci n) -> p ci n", n=N)
        src = xb[512 * b : 512 * (b + 1), :].rearrange("(ci p) n -> p ci n", p=128)
        eng = nc.sync if b % 2 == 0 else nc.gpsimd
        npc = 4 if b in (0, BPC - 1) else 2
        step = N // npc
        for q in range(npc):
            nsl = slice(q * step, (q + 1) * step)
            eng.dma_start(xn3[:, :, nsl], src[:, :, nsl])
        xns[b] = xn

    state = {}

    def stage_dots(b):
        """transposed dots: dotsT[n, h] per n-chunk j, PSUM chains over ci."""
        dt = dt_ps.tile([128, 512], F32, name=f"dt{b}", tag="dt")
        xn3 = xns[b][:].rearrange("p (ci n) -> p ci n", n=N)
        for j in range(NJ):
            o = dt[:, 8 * j : 8 * j + 8]
            # init with the (q . pe) term (has the -8 exp-shift folded in)
            nc.tensor.matmul(o, i128[:], peqT[:, 8 * j : 8 * j + 8],
                             start=True, stop=False)
            for ci in range(NCI):
                nc.tensor.matmul(
                    o,
                    xn3[:, ci, 128 * j : 128 * j + 128],
                    wqpe[:, 8 * ci : 8 * ci + 8],
                    start=False,
                    stop=(ci == NCI - 1),
                )
        state[b] = {"dt": dt}

    def stage_trans(b):
        """PE-transpose x into (n, c) tiles; Act/DVE alternate copying the
        PSUM tiles to SBUF bf16."""
        xn3 = xns[b][:].rearrange("p (ci n) -> p ci n", n=N)
        xts = xts_pool.tile([128, NJ * D], BF16, name=f"xts{b}", tag="xts")
        for j in range(NJ):
            xt = xt_ps.tile([128, D], F32, name=f"xt{b}_{j}", tag="xt")
            for ci in range(NCI):
                nc.tensor.matmul(
                    xt[:, 128 * ci : 128 * ci + 128],
                    xn3[:, ci, 128 * j : 128 * j + 128],
                    i128[:],
                    start=True,
                    stop=True,
                )
            dst = xts[:, D * j : D * (j + 1)]
            if j % 2 == 0:
                nc.scalar.copy(dst, xt[:])
            else:
                nc.vector.tensor_copy(dst, xt[:])
        state[b]["xts"] = xts

    def stage_exp(b):
        """exp(dotsT) -> attnT directly (shift folded into peqT)."""
        attnT = attn_pool.tile([128, 8 * NJ], BF16, name=f"attnT{b}", tag="attnT")
        # exp(dots - 8): 8 is a safe upper bound on the logits (observed max
        # ~4.3), so no max-reduce is needed; the shift cancels in
        # normalization.  Applied via fp32 bias (folding it into bf16 peqT
        # costs ~0.016 absolute per logit).
        nc.scalar.activation(
            attnT[:], state[b]["dt"][:, 0 : 8 * NJ],
            mybir.ActivationFunctionType.Exp,
            bias=nbias[:],
        )
        state[b]["attnT"] = attnT

    def stage_ssum(b):
        """softmax denominators via 1-col matmuls against ones."""
        dt, attnT = state[b]["dt"], state[b]["attnT"]
        for j in range(NJ):
            nc.tensor.matmul(
                dt[0:8, 64:65],
                attnT[:, 8 * j : 8 * j + 8],
                ones[:],
                start=(j == 0),
                stop=(j == NJ - 1),
            )

    def stage_rsum(b):
        # engines may not write at a partition offset, so recip lands in a
        # partition-0 tile and a tiny SBUF->SBUF DMA scatters it into place
        rs = attn_pool.tile([8, 1], F32, name=f"rs{b}", tag="rs")
        nc.vector.reciprocal(rs[:], state[b]["dt"][0:8, 64:65])
        nc.sync.dma_start(rsum_all[8 * b : 8 * b + 8, :], rs[:])

    def stage_pool(b):
        """sT[c, (ci,b,h)] += xT_tile^T @ attnT — 8-col matmuls, x stationary."""
        xts, attnT = state[b]["xts"], state[b]["attnT"]
        for ci in range(NCI):
            o = st_acc[:, 64 * ci + 8 * b : 64 * ci + 8 * b + 8]
            for j in range(NJ):
                nc.tensor.matmul(
                    o,
                    xts[:, D * j + 128 * ci : D * j + 128 * ci + 128],
                    attnT[:, 8 * j : 8 * j + 8],
                    start=(j == 0),
                    stop=(j == NJ - 1),
                )
        del state[b]

    st3 = st_acc[:].rearrange("p (ci q) -> p ci q", q=64)
    sb3 = stsb[:].rearrange("p (ci q) -> p ci q", q=64)

    def stage_tail_early():
        # batches 0-5 columns of s^T cast early; only b6/b7 left for the tail
        nc.scalar.copy(sb3[:, :, 0:48], st3[:, :, 0:48])

    def stage_tail():
        nc.scalar.copy(sb3[:, :, 48:64], st3[:, :, 48:64])
        for ci in range(NCI):
            nc.tensor.matmul(
                ops[:],
                stsb[:, 64 * ci : 64 * ci + 64],
                wv[:, D * ci : D * (ci + 1)],
                start=(ci == 0),
                stop=(ci == NCI - 1),
            )
        # out = ops * (1/sum) + bv, fused on DVE
        nc.vector.scalar_tensor_tensor(
            osb[:], ops[:], rsum_all[:], bvr[:],
            mybir.AluOpType.mult, mybir.AluOpType.add,
        )
        nc.sync.dma_start(t["out"], osb[:])

    # software pipeline; stage k of batch b emitted in iteration b + OFF[k]
    for i in range(BPC + 2):
        if i < BPC:
            stage_load(i)
        if i == BPC:
            nc.sync.dma_start(wv[:], t["wv"])
            nc.sync.dma_start(bvr[:], t["bvr"])
        if 2 <= i <= BPC + 1:
            stage_pool(i - 2)
        if 1 <= i <= BPC:
            b = i - 1
            stage_dots(b)
            stage_exp(b)
            stage_trans(b)
            stage_ssum(b)
            stage_rsum(b)
        if i == BPC:
            stage_tail_early()
        if i == BPC + 1:
            stage_tail()


_BUILT = None


def _build():
    global _BUILT
    if _BUILT is not None:
        return _BUILT
    nc = bacc.Bacc("TRN2", target_bir_lowering=False, debug=False)
    t = {
        "xb": nc.dram_tensor("xb", (BPC * D, N), BF16, kind="ExternalInput").ap(),
        "wqpe": nc.dram_tensor("wqpe", (128, 8 * NCI), BF16, kind="ExternalInput").ap(),
        "peqT": nc.dram_tensor("peqT", (128, 8 * NJ), BF16, kind="ExternalInput").ap(),
        "i128": nc.dram_tensor("i128", (128, 128), BF16, kind="ExternalInput").ap(),
        "wv": nc.dram_tensor("wv", (128, NCI * D), BF16, kind="ExternalInput").ap(),
        "bvr": nc.dram_tensor("bvr", (64, D), F32, kind="ExternalInput").ap(),
        "out": nc.dram_tensor("out", (64, D), F32, kind="ExternalOutput").ap(),
    }
    with tile.TileContext(nc) as tc:
        with ExitStack() as ctx:
            _emit(ctx, tc, t)
    nc.compile()
    _BUILT = (nc, t)
    return _BUILT


def _host_consts(q, Wkv, bkv):
    qh = np.asarray(q, np.float32)[0, :, 0, :]                      # (8, 64)
    Wk = np.asarray(Wkv, np.float32)[:, :D]
    Wv = np.asarray(Wkv, np.float32)[:, D:]
    bv = np.asarray(bkv, np.float32)[D:]

    position = np.arange(N, dtype=np.float32)[:, None]
    div_term = np.exp(
        np.arange(0, DH, 2, dtype=np.float32) * (-(math.log(10000.0) / DH))
    )
    pe = np.zeros((N, DH), np.float32)
    pe[:, 0::2] = np.sin(position * div_term)
    pe[:, 1::2] = np.cos(position * div_term)

    wq = np.einsum("chd,hd->ch", Wk.reshape(D, NH, DH), qh) * SCALE  # (512, 8)
    peq = pe @ (qh * SCALE).T                                        # (1024, 8)

    wqpe = np.zeros((128, 8 * NCI), np.float32)
    for ci in range(NCI):
        wqpe[:, 8 * ci : 8 * ci + 8] = wq[128 * ci : 128 * (ci + 1), :]
    peqT = np.zeros((128, 8 * NJ), np.float32)
    for j in range(NJ):
        peqT[:, 8 * j : 8 * j + 8] = peq[128 * j : 128 * (j + 1), :]

    wv_packed = np.zeros((128, NCI * D), np.float32)
    for ci in range(NCI):
        wv_packed[:, D * ci : D * (ci + 1)] = Wv[128 * ci : 128 * (ci + 1), :]

    return {
        "wqpe": wqpe.astype(ml_dtypes.bfloat16),
        "peqT": peqT.astype(ml_dtypes.bfloat16),
        "i128": np.eye(128, dtype=np.float32).astype(ml_dtypes.bfloat16),
        "wv": wv_packed.astype(ml_dtypes.bfloat16),
        "bvr": np.tile(bv, (64, 1)).astype(np.float32),
    }


def kernel(x, q, Wkv, bkv, num_heads, **kw):
    assert int(num_heads) == NH
    nc, _ = _build()
    consts = _host_consts(q, Wkv, bkv)

    xb = np.asarray(x, np.float32).reshape(B, D, N).astype(ml_dtypes.bfloat16)

    in_maps = []
    for i in range(NCORES):
        m = dict(consts)
        m["xb"] = np.ascontiguousarray(xb[i * BPC : (i + 1) * BPC]).reshape(BPC * D, N)
        in_maps.append(m)

    res = run_bass_kernel_spmd(nc, in_maps, core_ids=list(range(NCORES)))

    out = np.zeros((B, NH * DH), np.float32)
    hidx = np.arange(NH)
    for i in range(NCORES):
        shard = res.results[i]["out"].reshape(BPC, NH, NH, DH)[:, hidx, hidx, :]
        out[i * BPC : (i + 1) * BPC] = shard.reshape(BPC, NH * DH)
    return out


if __name__ == "__main__":
    _build()
    print("build ok")
